# revision 3
# baseline (speedup 1.0000x reference)
"""Trainium2 Bass kernel for nn_Correlation: stereo cost volume (25 disparities,
contraction over 256 channels) + 3x3 box filter, on inputs [16, 512, 64, 192] f32.

Strategy: data-parallel over batch (2 samples per core, 8 cores).
Per (b, h): banded Gram matmuls G[w, w'] on TensorE (only the w'-band
[w-24, w] is needed), staged to HBM scratch, then a diagonal-strided DMA
gather (stride BAND+1) pulls the 25 disparities per w as contiguous runs
into a [b*h, w, k] SBUF layout. Box filter = 2 shifted adds over w (DVE)
+ a block-tridiagonal matmul over the (b,h) partition dim (TensorE).
"""

import sys

if '/opt/trn_rl_repo' not in sys.path:
    sys.path.insert(0, '/opt/trn_rl_repo')

import numpy as np

import concourse.bass as bass
import concourse.mybir as mybir
from concourse import bacc
from concourse.tile import TileContext

F32 = mybir.dt.float32

# Problem constants (hardcoded per contest rules)
B, C2, H, W = 16, 512, 64, 192
C = C2 // 2          # 256 channels per image
MAXD = 24
D = MAXD + 1         # 25 disparities
NCORES = 8
B_PC = B // NCORES   # 2 samples per core

NCC = C // 128       # 2 contraction chunks
WBLK = 64            # Gram M-block size
NMB = W // WBLK      # 3 w-blocks
BAND = WBLK + MAXD   # 88 columns of x2 per block
W2P = W + MAXD       # padded x2 width (216)


def build_nc(b_pc=B_PC, h=H, hblk=8):
    """Build the per-core Bass program. Parameterized so a small config can be
    simulated quickly; the full config is (2, 64, 8)."""
    nhb = h // hblk
    pcorr = b_pc * h           # corr partition count (128 full)
    assert pcorr <= 128
    wpad = W + 2               # w-padded corr rows (zero at w=-1, w=W)
    fcorr = wpad * D

    nc = bacc.Bacc("TRN2", target_bir_lowering=False, debug=False)
    x = nc.dram_tensor("x", [b_pc, C2, h, W], F32, kind="ExternalInput")
    tmat = nc.dram_tensor("tmat", [pcorr, pcorr], F32, kind="ExternalInput")
    out = nc.dram_tensor("out", [b_pc, D, h, W], F32, kind="ExternalOutput")

    with TileContext(nc) as tc:
        with (
            tc.tile_pool(name="consts", bufs=1) as consts,
            tc.tile_pool(name="xin", bufs=2) as xin,
            tc.tile_pool(name="stg", bufs=2) as stg,
            tc.tile_pool(name="big", bufs=1) as big,
            tc.tile_pool(name="gpsum", bufs=4, space="PSUM") as gpsum,
            tc.tile_pool(name="bpsum", bufs=2, space="PSUM") as bpsum,
            tc.tile_pool(name="dram", bufs=2, space="DRAM") as dram,
        ):
            tmat_sb = consts.tile([pcorr, pcorr], F32)
            nc.sync.dma_start(tmat_sb[:], tmat.ap())

            corr = big.tile([pcorr, fcorr], F32)
            # zero the w = -1 and w = W pad rows
            nc.vector.memset(corr[:, 0:D], 0.0)
            nc.vector.memset(corr[:, (W + 1) * D:], 0.0)

            for b in range(b_pc):
                for hb in range(nhb):
                    x1t, x2t = [], []
                    for cc in range(NCC):
                        t1 = xin.tile([128, hblk, W], F32, tag=f"x1_{cc}")
                        nc.sync.dma_start(
                            t1[:],
                            x.ap()[b, cc * 128:(cc + 1) * 128,
                                   hb * hblk:(hb + 1) * hblk, :])
                        t2 = xin.tile([128, hblk, W2P], F32, tag=f"x2_{cc}")
                        nc.vector.memset(t2[:, :, 0:MAXD], 0.0)
                        nc.sync.dma_start(
                            t2[:, :, MAXD:],
                            x.ap()[b, C + cc * 128:C + (cc + 1) * 128,
                                   hb * hblk:(hb + 1) * hblk, :])
                        x1t.append(t1)
                        x2t.append(t2)

                    stage_sb = stg.tile([WBLK, hblk * NMB * BAND], F32, tag="ssb")
                    for hl in range(hblk):
                        psum_g = gpsum.tile([WBLK, NMB * BAND], F32, tag="g")
                        for m in range(NMB):
                            for cc in range(NCC):
                                nc.tensor.matmul(
                                    psum_g[:, m * BAND:(m + 1) * BAND],
                                    x1t[cc][:, hl, m * WBLK:(m + 1) * WBLK],
                                    x2t[cc][:, hl, m * WBLK:m * WBLK + BAND],
                                    start=(cc == 0), stop=(cc == NCC - 1))
                        evdst = stage_sb[:, hl * NMB * BAND:(hl + 1) * NMB * BAND]
                        if hl % 2 == 0:
                            nc.scalar.copy(evdst, psum_g[:])
                        else:
                            nc.vector.tensor_copy(out=evdst, in_=psum_g[:])

                    # stage the Gram bands to HBM scratch:
                    # layout stage[hl, m, p, j] at hl*(NMB*WBLK*BAND) + m*(WBLK*BAND) + p*BAND + j
                    stage_dr = dram.tile([hblk * NMB * WBLK * BAND], F32, tag="sdr")
                    sdr = stage_dr[:]
                    src = stage_sb[:, :].rearrange(
                        "p (hl m j) -> p hl m j", hl=hblk, m=NMB, j=BAND)
                    dst = bass.AP(sdr.tensor, sdr.offset, [
                        [BAND, WBLK],              # p
                        [NMB * WBLK * BAND, hblk],  # hl
                        [WBLK * BAND, NMB],        # m
                        [1, BAND],                 # j
                    ])
                    nc.sync.dma_start(dst, src)

                    # diagonal gather: corr[bh, w*D + k] = stage[hl, m, p, p + k]
                    gsrc = bass.AP(sdr.tensor, sdr.offset, [
                        [NMB * WBLK * BAND, hblk],  # hl
                        [WBLK * BAND, NMB],        # m
                        [BAND + 1, WBLK],          # p   (diagonal stride!)
                        [1, D],                    # k
                    ])
                    bh0 = b * h + hb * hblk
                    gdst = corr[bh0:bh0 + hblk, D:(W + 1) * D].rearrange(
                        "p (m q k) -> p m q k", m=NMB, q=WBLK, k=D)
                    nc.gpsimd.dma_start(gdst, gsrc)

            # --- box filter ---
            # w-direction: corrw[bh, w, k] = sum_{dw} corr[bh, w+dw, k]
            corrw = big.tile([pcorr, W * D], F32)
            nc.vector.tensor_tensor(
                out=corrw[:], in0=corr[:, 0:W * D], in1=corr[:, D:(W + 1) * D],
                op=mybir.AluOpType.add)
            nc.vector.tensor_tensor(
                out=corrw[:], in0=corrw[:], in1=corr[:, 2 * D:(W + 2) * D],
                op=mybir.AluOpType.add)

            # h-direction: block-tridiagonal matmul over the partition dim,
            # evicted with a (w,k) -> (k,w) transpose so the final writeback
            # DMAs are w-contiguous.
            out_sb = big.tile([pcorr, D * W], F32)
            cw = 20  # w-rows per chunk; 20*25=500 <= 512 psum limit
            chunks = []
            o = 0
            while o < W:
                nw = min(cw, W - o)
                chunks.append((o, nw))
                o += nw
            for ci, (o, nw) in enumerate(chunks):
                psum_b = bpsum.tile([pcorr, cw * D], F32, tag="b")
                nc.tensor.matmul(
                    psum_b[:, :nw * D], tmat_sb[:], corrw[:, o * D:(o + nw) * D],
                    start=True, stop=True)
                src = psum_b[:, :nw * D].rearrange("p (w k) -> p w k", w=nw)
                dst = out_sb[:, :].rearrange(
                    "p (k w) -> p k w", k=D)[:, :, o:o + nw].transpose([0, 2, 1])
                if ci % 2 == 0:
                    nc.scalar.copy(dst, src)
                else:
                    nc.vector.tensor_copy(out=dst, in_=src)

            # writeback: k -> disparity i = MAXD - k. Keep the SBUF partition
            # dim as a single AP dim; express the (b, h) split on the DRAM side.
            o_ap = out.ap()
            for k in range(D):
                src = out_sb[:, k * W:(k + 1) * W]
                dst = bass.AP(o_ap.tensor, (MAXD - k) * h * W, [
                    [D * h * W, b_pc],
                    [W, h],
                    [1, W],
                ])
                nc.sync.dma_start(dst, src)

    nc.compile()
    return nc


def make_tmat(b_pc=B_PC, h=H):
    pcorr = b_pc * h
    t = np.zeros((pcorr, pcorr), dtype=np.float32)
    for b in range(b_pc):
        for hh in range(h):
            for dh in (-1, 0, 1):
                if 0 <= hh + dh < h:
                    t[b * h + hh + dh, b * h + hh] = 1.0
    return t


_CACHE = {}


def kernel(inputs: np.ndarray) -> np.ndarray:
    """Full-input entry point: inputs [16, 512, 64, 192] f32 ->
    output [16, 25, 64, 192] f32."""
    from concourse.bass_utils import run_bass_kernel_spmd

    inputs = np.ascontiguousarray(inputs, dtype=np.float32)
    if 'nc' not in _CACHE:
        _CACHE['nc'] = build_nc()
        _CACHE['tmat'] = make_tmat()
    nc = _CACHE['nc']
    tm = _CACHE['tmat']

    in_maps = [
        {"x": np.ascontiguousarray(inputs[k * B_PC:(k + 1) * B_PC]), "tmat": tm}
        for k in range(NCORES)
    ]
    res = run_bass_kernel_spmd(nc, in_maps, core_ids=list(range(NCORES)))
    return np.concatenate([res.results[k]["out"] for k in range(NCORES)], axis=0)


if __name__ == "__main__":
    rng = np.random.default_rng(0)
    xs = rng.standard_normal((B, C2, H, W), dtype=np.float32)
    y = kernel(xs)
    print("out shape:", y.shape, y.dtype)
